# revision 30
# baseline (speedup 1.0000x reference)
"""Trainium2 Bass kernel for nn_Downsample_PASA_group_softmax (pooling).

Full-input contract: kernel(**inputs) takes the complete batch (n=8) and
returns the full output. Sharding: pure data parallelism, one sample per
NeuronCore across 8 cores (same Bass/Tile program, per-core in_maps).

Per-core pipeline v2:
  x arrives host-reflect-padded in pitch-130 fp16 rows, loaded into two
  channel-MIXED tiles (partitions = 64 group-0 + 64 group-1 channels), so
  every conv/pooling tap shift is a plain AP offset (no shifted copies).
  Conv3x3 -> 18 narrow accumulating matmuls per 2-row chunk (kh/kw
  shifts ride the rhs stream offset, z lands combined in PSUM); BN+exp on
  ScalarE reading the pitched interior; softmax denom via ones matmul (x1/256) + fast-approx
  reciprocal; sigma broadcast 18->128 with 9 merged-group selector
  matmuls (each serves both groups, halving bcast+evac); ScalarE
  evacuates PSUM; the 34 pooling mult/add passes all run on DVE (fp16
  2x), unit-interleaved with the next super-block's conv/bcast emission.
  Pool engine offload was measured net-negative (shared SBUF ports).
  Measured ~442 us/core steady-state on HW (baseline 580).
"""

import numpy as np
from contextlib import ExitStack

import concourse.mybir as mybir
from concourse.ap import AP

N_CORES = 8

FP16 = mybir.dt.float16
FP32 = mybir.dt.float32
AF = mybir.ActivationFunctionType
ALU = mybir.AluOpType

C = 256
H = W = 128
Q = H * W              # 16384 pixels
G = 2
K = 3
NK = K * K             # 9
NO = G * NK            # 18 conv outputs
PW = W + 2             # padded row pitch (col -1 and 128 reflect)
PR = H + 2             # padded rows (row -1 and 128 reflect)
XPLEN = PR * PW + 2    # 130*130 (+2 tail so kw=2 streams stay in bounds)

SB_ROWS = 16           # super-block rows
N_SB = H // SB_ROWS    # 8
SPAN = SB_ROWS * W     # 2048 compact px per sb (output/DMA)
SP = SB_ROWS * PW      # 2080 pitched px per sb (on-chip sigma/product layout)
CHUNK_ROWS = 2
CHUNK = CHUNK_ROWS * W           # 512 compact px per conv chunk
N_CH = SB_ROWS // CHUNK_ROWS     # 4 conv chunks per sb
CSTREAM = CHUNK_ROWS * PW        # 520 pitched rhs cols per conv chunk
EV = 2 * CSTREAM       # bcast/evac piece: 1040 pitched px


def _mix(cblk):
    """Channel list for mixed tile cblk: 64 group-0 + 64 group-1 channels."""
    lo = [cblk * 64 + i for i in range(64)]
    hi = [128 + cblk * 64 + i for i in range(64)]
    return lo + hi


def host_constants(conv_w, gamma, beta, run_mean, run_var):
    w = np.asarray(conv_w, np.float32)  # (18, 256, 3, 3)
    # narrow conv lhsT: 18 blocks of [128, 18], one per (cblk, kh, kw); the
    # kh/kw shifts ride the rhs stream offset (pitch-130 layout), so all 18
    # matmuls accumulate the combined conv sum z directly in PSUM
    lhsT_conv = np.zeros((128, 18 * NO), np.float16)
    for cb in range(2):
        chans = _mix(cb)
        for kh in range(K):
            for kw in range(K):
                m = (cb * K + kh) * K + kw
                lhsT_conv[:, m * NO:(m + 1) * NO] = w[:, chans, kh, kw].T.astype(np.float16)
    # merged-group selector: per tap k a [18, 128] block;
    # partitions 0:64 take sigma row k (group 0), 64:128 take row 9+k.
    sel = np.zeros((NO, NK * 128), np.float16)
    for k in range(NK):
        sel[k, k * 128:k * 128 + 64] = 1.0
        sel[NK + k, k * 128 + 64:(k + 1) * 128] = 1.0
    ones18 = np.full((NO, NO), 1.0 / 256.0, np.float16)
    scale = np.asarray(gamma, np.float32) / np.sqrt(np.asarray(run_var, np.float32) + 1e-5)
    bias = np.asarray(beta, np.float32) - np.asarray(run_mean, np.float32) * scale
    return {
        "lhsT_conv": lhsT_conv,
        "sel": sel,
        "ones18": ones18,
        "bn_scale": scale.reshape(NO, 1).astype(np.float32),
        "bn_bias": bias.reshape(NO, 1).astype(np.float32),
    }


def declare_io(nc):
    ins = {
        "x": nc.dram_tensor("x", (C, XPLEN), FP16, kind="ExternalInput").ap(),
        "lhsT_conv": nc.dram_tensor("lhsT_conv", (128, 18 * NO), FP16, kind="ExternalInput").ap(),
        "sel": nc.dram_tensor("sel", (NO, NK * 128), FP16, kind="ExternalInput").ap(),
        "ones18": nc.dram_tensor("ones18", (NO, NO), FP16, kind="ExternalInput").ap(),
        "bn_scale": nc.dram_tensor("bn_scale", (NO, 1), FP32, kind="ExternalInput").ap(),
        "bn_bias": nc.dram_tensor("bn_bias", (NO, 1), FP32, kind="ExternalInput").ap(),
    }
    out = nc.dram_tensor("out", (C, Q), FP16, kind="ExternalOutput").ap()
    return ins, out


def make_pools(ctx: ExitStack, tc):
    p = {}
    p["const"] = ctx.enter_context(tc.tile_pool(name="const", bufs=1))
    p["xp"] = ctx.enter_context(tc.tile_pool(name="xp", bufs=1))
    p["e"] = ctx.enter_context(tc.tile_pool(name="e", bufs=1))
    p["rchunk"] = ctx.enter_context(tc.tile_pool(name="rchunk", bufs=1))
    p["sgb"] = ctx.enter_context(tc.tile_pool(name="sgb", bufs=2))
    p["pr1"] = ctx.enter_context(tc.tile_pool(name="pr1", bufs=1))
    p["psc"] = ctx.enter_context(tc.tile_pool(name="psc", bufs=2, space="PSUM"))
    p["psb"] = ctx.enter_context(tc.tile_pool(name="psb", bufs=2, space="PSUM"))
    p["psd"] = ctx.enter_context(tc.tile_pool(name="psd", bufs=2, space="PSUM"))
    return p


def load_consts(tc, p, in_aps):
    nc = tc.nc
    const = p["const"]
    c = {}
    for name, shape, dt in (
        ("lhsT_conv", [128, 18 * NO], FP16),
        ("sel", [NO, NK * 128], FP16),
        ("ones18", [NO, NO], FP16),
        ("bn_scale", [NO, 1], FP32),
        ("bn_bias", [NO, 1], FP32),
    ):
        c[name] = const.tile(shape, dt, tag=name, name=name)
        nc.sync.dma_start(c[name][:], in_aps[name][:])
    return c


def emit_body(tc, p, c, out_ap, in_aps):
    nc = tc.nc
    x_d = in_aps["x"]
    lhsT_conv, sel, ones18 = c["lhsT_conv"], c["sel"], c["ones18"]
    bn_scale, bn_bias = c["bn_scale"], c["bn_bias"]

    # ---- x: two channel-mixed pitch-130 padded fp16 tiles ----
    # pos(r, col) = (r+1)*PW + (col+1), r in -1..128, col in -1..128
    xp = []
    for cb in range(2):
        t = p["xp"].tile([128, XPLEN], FP16, tag=f"xp{cb}")
        xp.append(t)
        # host-padded pitch-130 rows: one contiguous DMA per channel block
        nc.sync.dma_start(t[0:64, :], x_d[cb * 64:cb * 64 + 64, :])
        nc.sync.dma_start(t[64:128, :], x_d[128 + cb * 64:128 + cb * 64 + 64, :])

    def emit_conv_chunk(sb, cc, E, den_t):
        """Conv+BN+exp+denominator for 2 rows (520 pitched px).

        The whole sigma pipeline stays in the pitched (row-pitch PW) layout;
        the 2 pad columns per row carry finite garbage that is never DMA'd.
        """
        r0 = sb * SB_ROWS + cc * CHUNK_ROWS
        eq0 = cc * CSTREAM
        cps = p["psc"].tile([NO, CSTREAM], FP32, tag="conv", name="cps")
        for m in range(18):
            cbkh, kw = divmod(m, 3)
            cb, kh = divmod(cbkh, 3)
            base = r0 * PW + kh * PW + kw  # pos(r0+rr-1+kh, col+kw-1) - (rr*PW+col)
            nc.tensor.matmul(
                cps[:],
                lhsT_conv[:, m * NO:(m + 1) * NO],
                xp[cb][:, base:base + CSTREAM],
                start=(m == 0),
                stop=(m == 17),
            )
        # BN + exp -> E chunk (fp16), flat pitched read of conv PSUM
        nc.scalar.activation(E[:, eq0:eq0 + CSTREAM], cps[:], AF.Exp,
                             bias=bn_bias[:], scale=bn_scale[:])
        nc.tensor.matmul(den_t[:], ones18[:],
                         E[:, eq0:eq0 + CSTREAM], start=True, stop=True)
        rch = p["rchunk"].tile([NO, CSTREAM], FP32, tag="r", name="rch")
        with nc.allow_low_precision("softmax recip in fp16"):
            nc.vector.reciprocal_approx_fast(rch[:], den_t[:])
        nc.vector.scalar_tensor_tensor(
            E[:, eq0:eq0 + CSTREAM], E[:, eq0:eq0 + CSTREAM], 1.0 / 256.0,
            rch[:], ALU.mult, ALU.mult,
        )

    def emit_bcast_unit(st, k, piece):
        """Broadcast sigma tap k (both groups) piece -> sgb[k] via PE+ScalarE.

        Each 260-col matmul lands at a bank-aligned 512-fp32 offset in PSUM
        (a matmul output must not cross a 2KB PSUM bank boundary); the evac
        re-packs the two segments contiguously into sgb.
        """
        E, sgb = st["E"], st["sgb"]
        bps = p["psb"].tile([128, 1024], FP32, tag="b", name="bps")
        for j in range(EV // CSTREAM):
            qq = piece * EV + j * CSTREAM
            nc.tensor.matmul(bps[:, j * 512:j * 512 + CSTREAM],
                             sel[:, k * 128:(k + 1) * 128],
                             E[:, qq:qq + CSTREAM], start=True, stop=True)
        b = bps[:]
        src = AP(b.tensor, b.offset, [[b.ap[0][0], 128], [512, 2], [1, CSTREAM]])
        dst = sgb[:, k * SP + piece * EV:k * SP + (piece + 1) * EV]
        nc.scalar.copy(dst.rearrange("p (a b) -> p a b", a=2), src)

    def xvk(cb, sb, k):
        """Flat pitched-sb window of xp[cb] for tap k (contiguous [128, SP])."""
        kh, kw = divmod(k, K)
        base = xp[cb][:]
        pstride = base.ap[0][0]
        return AP(base.tensor, base.offset + (sb * SB_ROWS + kh) * PW + kw,
                  [[pstride, 128], [1, SP]])

    def pool_units(st):
        """Per-tap in-place products (tap k ready as soon as its evacs land),
        then merged 4-op tree adds per tile, then out DMA."""
        sb = st["sb"]
        sgb = st["sgb"]

        def mul(cb, k):
            if cb == 1:
                if "pr1" not in st:
                    st["pr1"] = p["pr1"].tile([128, NK * SP], FP16, tag="pr1",
                                              name="pr1")
                nc.vector.tensor_mul(st["pr1"][:, k * SP:(k + 1) * SP],
                                     sgb[:, k * SP:(k + 1) * SP], xvk(1, sb, k))
            else:
                nc.vector.tensor_mul(sgb[:, k * SP:(k + 1) * SP],
                                     sgb[:, k * SP:(k + 1) * SP], xvk(0, sb, k))

        def tree(which, lvl):
            t = st["pr1"] if which else sgb
            if lvl == 0:
                nc.vector.tensor_add(t[:, 0:4 * SP], t[:, 0:4 * SP],
                                     t[:, 4 * SP:8 * SP])
            elif lvl == 1:
                nc.vector.tensor_add(t[:, 0:2 * SP], t[:, 0:2 * SP],
                                     t[:, 2 * SP:4 * SP])
            elif lvl == 2:
                nc.vector.tensor_add(t[:, 0:SP], t[:, 0:SP], t[:, SP:2 * SP])
            else:
                nc.vector.tensor_add(t[:, 0:SP], t[:, 0:SP], t[:, 8 * SP:9 * SP])

        # tap 0 last: slot 0 is the tree/out-DMA region of the PREVIOUS sb's
        # pr1 (bufs=1), so touching it first would stall on the out DMA.
        for k in list(range(1, NK)) + [0]:
            yield lambda k=k: mul(1, k)
            yield lambda k=k: mul(0, k)
        for lvl in range(4):
            yield lambda lvl=lvl: tree(1, lvl)
            yield lambda lvl=lvl: tree(0, lvl)
        yield lambda: emit_out(st, 1)
        yield lambda: emit_out(st, 0)

    def emit_out(st, cb):
        """DMA the pooled sb out, skipping the 2 pad columns per pitched row."""
        sb = st["sb"]
        t = st["sgb"] if cb == 0 else st["pr1"]
        base = t[:]
        pstride = base.ap[0][0]
        q0 = sb * SPAN
        dst = out_ap.rearrange("(blk grp ch) q -> blk grp ch q", blk=2, grp=2)
        for blk in range(2):
            src = AP(base.tensor, base.offset + blk * 64 * pstride,
                     [[pstride, 64], [PW, SB_ROWS], [1, W]])
            d = dst[blk, cb, :, q0:q0 + SPAN].rearrange("ch (r j) -> ch r j", j=W)
            nc.sync.dma_start(d, src)

    def make_sb_state(sb):
        E = p["e"].tile([NO, SP], FP16, tag="e", name="E")
        sgb = p["sgb"].tile([128, NK * SP], FP16, tag="sgb", name="sgb")
        return {"sb": sb, "E": E, "sgb": sgb}

    # ---- software-pipelined emission over super-blocks ----
    # per sb: Pool chain of prev launches first, then conv chunks + bcast of
    # sb (PE/Act/DVE), then prev's DVE pooling, so Pool and DVE overlap.
    def drain(it, n):
        done = 0
        for fn in it:
            fn()
            done += 1
            if done >= n:
                return
        return

    prev_units = iter(())
    for sb in range(N_SB):
        st = make_sb_state(sb)
        for cc in range(N_CH):
            den_t = p["psd"].tile([NO, CSTREAM], FP32, tag="den", name="den")
            emit_conv_chunk(sb, cc, st["E"], den_t)
            drain(prev_units, 2)
        for k in range(NK):
            for piece in range(SP // EV):
                emit_bcast_unit(st, k, piece)
            drain(prev_units, 1)
        for fn in prev_units:
            fn()
        prev_units = pool_units(st)
    for fn in prev_units:
        fn()


def build(ctx: ExitStack, tc, out_ap, in_aps):
    p = make_pools(ctx, tc)
    c = load_consts(tc, p, in_aps)
    emit_body(tc, p, c, out_ap, in_aps)


_COMPILED = {}


def _get_compiled():
    if "nc" not in _COMPILED:
        import concourse.bacc as bacc
        import concourse.tile as tile

        nc = bacc.Bacc("TRN2", target_bir_lowering=False, debug=False,
                       num_devices=N_CORES)
        ins, out_ap = declare_io(nc)
        with tile.TileContext(nc) as tc:
            with ExitStack() as ctx:
                build(ctx, tc, out_ap, ins)
        nc.compile()
        _COMPILED["nc"] = nc
    return _COMPILED["nc"]


def host_x(x_sample):
    """Reflect-pad one sample to the pitch-130 on-chip layout (fp16)."""
    xs = np.asarray(x_sample, np.float32).reshape(C, H, W)
    xpad = np.pad(xs, ((0, 0), (1, 1), (1, 1)), mode="reflect")
    flat = np.zeros((C, XPLEN), np.float16)
    flat[:, :PR * PW] = xpad.astype(np.float16).reshape(C, PR * PW)
    return flat


def kernel(x, conv_w, gamma, beta, run_mean, run_var):
    from concourse import bass_utils

    x = np.asarray(x, np.float32)
    n = x.shape[0]
    assert n == N_CORES, f"expected batch {N_CORES}, got {n}"
    consts = host_constants(np.asarray(conv_w, np.float32), np.asarray(gamma, np.float32),
                            np.asarray(beta, np.float32), np.asarray(run_mean, np.float32),
                            np.asarray(run_var, np.float32))
    nc = _get_compiled()
    in_maps = [{"x": host_x(x[i]), **consts} for i in range(N_CORES)]
    res = bass_utils.run_bass_kernel_spmd(nc, in_maps, core_ids=list(range(N_CORES)))
    out = np.stack([res.results[i]["out"].reshape(C, H, W) for i in range(N_CORES)])
    return out.astype(np.float32)



# revision 31
# speedup vs baseline: 1.0300x; 1.0300x over previous
"""Trainium2 Bass kernel for nn_Downsample_PASA_group_softmax (pooling).

Full-input contract: kernel(**inputs) takes the complete batch (n=8) and
returns the full output. Sharding: pure data parallelism, one sample per
NeuronCore across 8 cores (same Bass/Tile program, per-core in_maps).

Per-core pipeline v2:
  x arrives host-reflect-padded in pitch-130 fp16 rows, loaded into two
  channel-MIXED tiles (partitions = 64 group-0 + 64 group-1 channels), so
  every conv/pooling tap shift is a plain AP offset (no shifted copies).
  Conv3x3 -> 18 narrow accumulating matmuls per 2-row chunk (kh/kw
  shifts ride the rhs stream offset, z lands combined in PSUM); BN+exp on
  ScalarE reading the pitched interior; softmax denom via ones matmul (x1/256) + fast-approx
  reciprocal; sigma broadcast 18->128 with 9 merged-group selector
  matmuls (each serves both groups, halving bcast+evac); ScalarE
  evacuates PSUM; the 34 pooling mult/add passes all run on DVE (fp16
  2x), unit-interleaved with the next super-block's conv/bcast emission.
  Pool engine offload was measured net-negative (shared SBUF ports).
  Measured ~442 us/core steady-state on HW (baseline 580).
"""

import numpy as np
from contextlib import ExitStack

import concourse.mybir as mybir
from concourse.ap import AP

N_CORES = 8

FP16 = mybir.dt.float16
FP32 = mybir.dt.float32
AF = mybir.ActivationFunctionType
ALU = mybir.AluOpType

C = 256
H = W = 128
Q = H * W              # 16384 pixels
G = 2
K = 3
NK = K * K             # 9
NO = G * NK            # 18 conv outputs
PW = W + 2             # padded row pitch (col -1 and 128 reflect)
PR = H + 2             # padded rows (row -1 and 128 reflect)
XPLEN = PR * PW + 2    # 130*130 (+2 tail so kw=2 streams stay in bounds)

SB_ROWS = 16           # super-block rows
N_SB = H // SB_ROWS    # 8
SPAN = SB_ROWS * W     # 2048 compact px per sb (output/DMA)
SP = SB_ROWS * PW      # 2080 pitched px per sb (on-chip sigma/product layout)
CHUNK_ROWS = 2
CHUNK = CHUNK_ROWS * W           # 512 compact px per conv chunk
N_CH = SB_ROWS // CHUNK_ROWS     # 4 conv chunks per sb
CSTREAM = CHUNK_ROWS * PW        # 520 pitched rhs cols per conv chunk
EV = 2 * CSTREAM       # bcast/evac piece: 1040 pitched px


def _mix(cblk):
    """Channel list for mixed tile cblk: 64 group-0 + 64 group-1 channels."""
    lo = [cblk * 64 + i for i in range(64)]
    hi = [128 + cblk * 64 + i for i in range(64)]
    return lo + hi


def host_constants(conv_w, gamma, beta, run_mean, run_var):
    w = np.asarray(conv_w, np.float32)  # (18, 256, 3, 3)
    # narrow conv lhsT: 18 blocks of [128, 18], one per (cblk, kh, kw); the
    # kh/kw shifts ride the rhs stream offset (pitch-130 layout), so all 18
    # matmuls accumulate the combined conv sum z directly in PSUM
    lhsT_conv = np.zeros((128, 18 * NO), np.float16)
    for cb in range(2):
        chans = _mix(cb)
        for kh in range(K):
            for kw in range(K):
                m = (cb * K + kh) * K + kw
                lhsT_conv[:, m * NO:(m + 1) * NO] = w[:, chans, kh, kw].T.astype(np.float16)
    # merged-group selector: per tap k a [18, 128] block;
    # partitions 0:64 take sigma row k (group 0), 64:128 take row 9+k.
    sel = np.zeros((NO, NK * 128), np.float16)
    for k in range(NK):
        sel[k, k * 128:k * 128 + 64] = 1.0
        sel[NK + k, k * 128 + 64:(k + 1) * 128] = 1.0
    ones18 = np.full((NO, NO), 1.0 / 256.0, np.float16)
    scale = np.asarray(gamma, np.float32) / np.sqrt(np.asarray(run_var, np.float32) + 1e-5)
    bias = np.asarray(beta, np.float32) - np.asarray(run_mean, np.float32) * scale
    return {
        "lhsT_conv": lhsT_conv,
        "sel": sel,
        "ones18": ones18,
        "bn_scale": scale.reshape(NO, 1).astype(np.float32),
        "bn_bias": bias.reshape(NO, 1).astype(np.float32),
    }


def declare_io(nc):
    ins = {
        "x": nc.dram_tensor("x", (C, XPLEN), FP16, kind="ExternalInput").ap(),
        "lhsT_conv": nc.dram_tensor("lhsT_conv", (128, 18 * NO), FP16, kind="ExternalInput").ap(),
        "sel": nc.dram_tensor("sel", (NO, NK * 128), FP16, kind="ExternalInput").ap(),
        "ones18": nc.dram_tensor("ones18", (NO, NO), FP16, kind="ExternalInput").ap(),
        "bn_scale": nc.dram_tensor("bn_scale", (NO, 1), FP32, kind="ExternalInput").ap(),
        "bn_bias": nc.dram_tensor("bn_bias", (NO, 1), FP32, kind="ExternalInput").ap(),
    }
    out = nc.dram_tensor("out", (C, N_SB * SP), FP16, kind="ExternalOutput").ap()
    return ins, out


def make_pools(ctx: ExitStack, tc):
    p = {}
    p["const"] = ctx.enter_context(tc.tile_pool(name="const", bufs=1))
    p["xp"] = ctx.enter_context(tc.tile_pool(name="xp", bufs=1))
    p["e"] = ctx.enter_context(tc.tile_pool(name="e", bufs=1))
    p["rchunk"] = ctx.enter_context(tc.tile_pool(name="rchunk", bufs=1))
    p["sgb"] = ctx.enter_context(tc.tile_pool(name="sgb", bufs=2))
    p["pr1"] = ctx.enter_context(tc.tile_pool(name="pr1", bufs=1))
    p["psc"] = ctx.enter_context(tc.tile_pool(name="psc", bufs=2, space="PSUM"))
    p["psb"] = ctx.enter_context(tc.tile_pool(name="psb", bufs=2, space="PSUM"))
    p["psd"] = ctx.enter_context(tc.tile_pool(name="psd", bufs=2, space="PSUM"))
    return p


def load_consts(tc, p, in_aps):
    nc = tc.nc
    const = p["const"]
    c = {}
    for name, shape, dt in (
        ("lhsT_conv", [128, 18 * NO], FP16),
        ("sel", [NO, NK * 128], FP16),
        ("ones18", [NO, NO], FP16),
        ("bn_scale", [NO, 1], FP32),
        ("bn_bias", [NO, 1], FP32),
    ):
        c[name] = const.tile(shape, dt, tag=name, name=name)
        nc.sync.dma_start(c[name][:], in_aps[name][:])
    return c


def emit_body(tc, p, c, out_ap, in_aps):
    nc = tc.nc
    x_d = in_aps["x"]
    lhsT_conv, sel, ones18 = c["lhsT_conv"], c["sel"], c["ones18"]
    bn_scale, bn_bias = c["bn_scale"], c["bn_bias"]

    # ---- x: two channel-mixed pitch-130 padded fp16 tiles ----
    # pos(r, col) = (r+1)*PW + (col+1), r in -1..128, col in -1..128
    xp = []
    for cb in range(2):
        t = p["xp"].tile([128, XPLEN], FP16, tag=f"xp{cb}")
        xp.append(t)
        # host-padded pitch-130 rows: one contiguous DMA per channel block
        nc.sync.dma_start(t[0:64, :], x_d[cb * 64:cb * 64 + 64, :])
        nc.sync.dma_start(t[64:128, :], x_d[128 + cb * 64:128 + cb * 64 + 64, :])

    def emit_conv_chunk(sb, cc, E, den_t):
        """Conv+BN+exp+denominator for 2 rows (520 pitched px).

        The whole sigma pipeline stays in the pitched (row-pitch PW) layout;
        the 2 pad columns per row carry finite garbage that is never DMA'd.
        """
        r0 = sb * SB_ROWS + cc * CHUNK_ROWS
        eq0 = cc * CSTREAM
        cps = p["psc"].tile([NO, CSTREAM], FP32, tag="conv", name="cps")
        for m in range(18):
            cbkh, kw = divmod(m, 3)
            cb, kh = divmod(cbkh, 3)
            base = r0 * PW + kh * PW + kw  # pos(r0+rr-1+kh, col+kw-1) - (rr*PW+col)
            nc.tensor.matmul(
                cps[:],
                lhsT_conv[:, m * NO:(m + 1) * NO],
                xp[cb][:, base:base + CSTREAM],
                start=(m == 0),
                stop=(m == 17),
            )
        # BN + exp -> E chunk (fp16), flat pitched read of conv PSUM
        nc.scalar.activation(E[:, eq0:eq0 + CSTREAM], cps[:], AF.Exp,
                             bias=bn_bias[:], scale=bn_scale[:])
        nc.tensor.matmul(den_t[:], ones18[:],
                         E[:, eq0:eq0 + CSTREAM], start=True, stop=True)
        rch = p["rchunk"].tile([NO, CSTREAM], FP32, tag="r", name="rch")
        with nc.allow_low_precision("softmax recip in fp16"):
            nc.vector.reciprocal_approx_fast(rch[:], den_t[:])
        nc.vector.scalar_tensor_tensor(
            E[:, eq0:eq0 + CSTREAM], E[:, eq0:eq0 + CSTREAM], 1.0 / 256.0,
            rch[:], ALU.mult, ALU.mult,
        )

    def emit_bcast_unit(st, k, piece):
        """Broadcast sigma tap k (both groups) piece -> sgb[k] via PE+ScalarE.

        Each 260-col matmul lands at a bank-aligned 512-fp32 offset in PSUM
        (a matmul output must not cross a 2KB PSUM bank boundary); the evac
        re-packs the two segments contiguously into sgb.
        """
        E, sgb = st["E"], st["sgb"]
        bps = p["psb"].tile([128, 1024], FP32, tag="b", name="bps")
        for j in range(EV // CSTREAM):
            qq = piece * EV + j * CSTREAM
            nc.tensor.matmul(bps[:, j * 512:j * 512 + CSTREAM],
                             sel[:, k * 128:(k + 1) * 128],
                             E[:, qq:qq + CSTREAM], start=True, stop=True)
        b = bps[:]
        src = AP(b.tensor, b.offset, [[b.ap[0][0], 128], [512, 2], [1, CSTREAM]])
        dst = sgb[:, k * SP + piece * EV:k * SP + (piece + 1) * EV]
        nc.scalar.copy(dst.rearrange("p (a b) -> p a b", a=2), src)

    def xvk(cb, sb, k):
        """Flat pitched-sb window of xp[cb] for tap k (contiguous [128, SP])."""
        kh, kw = divmod(k, K)
        base = xp[cb][:]
        pstride = base.ap[0][0]
        return AP(base.tensor, base.offset + (sb * SB_ROWS + kh) * PW + kw,
                  [[pstride, 128], [1, SP]])

    def pool_units(st):
        """Per-tap in-place products (tap k ready as soon as its evacs land),
        then merged 4-op tree adds per tile, then out DMA."""
        sb = st["sb"]
        sgb = st["sgb"]

        def mul(cb, k):
            if cb == 1:
                if "pr1" not in st:
                    st["pr1"] = p["pr1"].tile([128, NK * SP], FP16, tag="pr1",
                                              name="pr1")
                nc.vector.tensor_mul(st["pr1"][:, k * SP:(k + 1) * SP],
                                     sgb[:, k * SP:(k + 1) * SP], xvk(1, sb, k))
            else:
                nc.vector.tensor_mul(sgb[:, k * SP:(k + 1) * SP],
                                     sgb[:, k * SP:(k + 1) * SP], xvk(0, sb, k))

        def tree(which, lvl):
            t = st["pr1"] if which else sgb
            if lvl == 0:
                nc.vector.tensor_add(t[:, 0:4 * SP], t[:, 0:4 * SP],
                                     t[:, 4 * SP:8 * SP])
            elif lvl == 1:
                nc.vector.tensor_add(t[:, 0:2 * SP], t[:, 0:2 * SP],
                                     t[:, 2 * SP:4 * SP])
            elif lvl == 2:
                nc.vector.tensor_add(t[:, 0:SP], t[:, 0:SP], t[:, SP:2 * SP])
            else:
                nc.vector.tensor_add(t[:, 0:SP], t[:, 0:SP], t[:, 8 * SP:9 * SP])

        # tap 0 last: slot 0 is the tree/out-DMA region of the PREVIOUS sb's
        # pr1 (bufs=1), so touching it first would stall on the out DMA.
        for k in list(range(1, NK)) + [0]:
            yield lambda k=k: mul(1, k)
            yield lambda k=k: mul(0, k)
        for lvl in range(4):
            yield lambda lvl=lvl: tree(1, lvl)
            yield lambda lvl=lvl: tree(0, lvl)
        yield lambda: emit_out(st, 1)
        yield lambda: emit_out(st, 0)

    def emit_out(st, cb):
        """DMA the pooled sb out in the pitched layout (host strips pads)."""
        sb = st["sb"]
        t = st["sgb"] if cb == 0 else st["pr1"]
        q0 = sb * SP
        dst = out_ap.rearrange("(blk grp ch) q -> blk grp ch q", blk=2, grp=2)
        nc.sync.dma_start(dst[0, cb, :, q0:q0 + SP], t[0:64, 0:SP])
        nc.sync.dma_start(dst[1, cb, :, q0:q0 + SP], t[64:128, 0:SP])

    def make_sb_state(sb):
        E = p["e"].tile([NO, SP], FP16, tag="e", name="E")
        sgb = p["sgb"].tile([128, NK * SP], FP16, tag="sgb", name="sgb")
        return {"sb": sb, "E": E, "sgb": sgb}

    # ---- software-pipelined emission over super-blocks ----
    # per sb: Pool chain of prev launches first, then conv chunks + bcast of
    # sb (PE/Act/DVE), then prev's DVE pooling, so Pool and DVE overlap.
    def drain(it, n):
        done = 0
        for fn in it:
            fn()
            done += 1
            if done >= n:
                return
        return

    prev_units = iter(())
    for sb in range(N_SB):
        st = make_sb_state(sb)
        for cc in range(N_CH):
            den_t = p["psd"].tile([NO, CSTREAM], FP32, tag="den", name="den")
            emit_conv_chunk(sb, cc, st["E"], den_t)
            drain(prev_units, 2)
        for k in range(NK):
            for piece in range(SP // EV):
                emit_bcast_unit(st, k, piece)
            drain(prev_units, 1)
        for fn in prev_units:
            fn()
        prev_units = pool_units(st)
    for fn in prev_units:
        fn()


def build(ctx: ExitStack, tc, out_ap, in_aps):
    p = make_pools(ctx, tc)
    c = load_consts(tc, p, in_aps)
    emit_body(tc, p, c, out_ap, in_aps)


_COMPILED = {}


def _get_compiled():
    if "nc" not in _COMPILED:
        import concourse.bacc as bacc
        import concourse.tile as tile

        nc = bacc.Bacc("TRN2", target_bir_lowering=False, debug=False,
                       num_devices=N_CORES)
        ins, out_ap = declare_io(nc)
        with tile.TileContext(nc) as tc:
            with ExitStack() as ctx:
                build(ctx, tc, out_ap, ins)
        nc.compile()
        _COMPILED["nc"] = nc
    return _COMPILED["nc"]


def host_x(x_sample):
    """Reflect-pad one sample to the pitch-130 on-chip layout (fp16)."""
    xs = np.asarray(x_sample, np.float32).reshape(C, H, W)
    xpad = np.pad(xs, ((0, 0), (1, 1), (1, 1)), mode="reflect")
    flat = np.zeros((C, XPLEN), np.float16)
    flat[:, :PR * PW] = xpad.astype(np.float16).reshape(C, PR * PW)
    return flat


def kernel(x, conv_w, gamma, beta, run_mean, run_var):
    from concourse import bass_utils

    x = np.asarray(x, np.float32)
    n = x.shape[0]
    assert n == N_CORES, f"expected batch {N_CORES}, got {n}"
    consts = host_constants(np.asarray(conv_w, np.float32), np.asarray(gamma, np.float32),
                            np.asarray(beta, np.float32), np.asarray(run_mean, np.float32),
                            np.asarray(run_var, np.float32))
    nc = _get_compiled()
    in_maps = [{"x": host_x(x[i]), **consts} for i in range(N_CORES)]
    res = bass_utils.run_bass_kernel_spmd(nc, in_maps, core_ids=list(range(N_CORES)))
    out = np.stack([
        res.results[i]["out"].reshape(C, N_SB, SB_ROWS, PW)[:, :, :, 0:W]
        .reshape(C, H, W)
        for i in range(N_CORES)
    ])
    return out.astype(np.float32)



# revision 32
# speedup vs baseline: 1.2014x; 1.1665x over previous
"""Trainium2 Bass kernel for nn_Downsample_PASA_group_softmax (pooling).

Full-input contract: kernel(**inputs) takes the complete batch (n=8) and
returns the full output. Sharding: pure data parallelism, one sample per
NeuronCore across 8 cores (same Bass/Tile program, per-core in_maps).

Per-core pipeline v2:
  x arrives host-reflect-padded in pitch-130 fp16 rows, loaded into two
  channel-MIXED tiles (partitions = 64 group-0 + 64 group-1 channels), so
  every conv/pooling tap shift is a plain AP offset (no shifted copies).
  Conv3x3 -> 18 narrow accumulating matmuls per 2-row chunk (kh/kw
  shifts ride the rhs stream offset, z lands combined in PSUM); BN+exp on
  ScalarE reading the pitched interior; softmax denom via ones matmul (x1/256) + fast-approx
  reciprocal; sigma broadcast 18->128 with 9 merged-group selector
  matmuls (each serves both groups, halving bcast+evac); ScalarE
  evacuates PSUM; the 34 pooling mult/add passes all run on DVE (fp16
  2x), unit-interleaved with the next super-block's conv/bcast emission.
  Pool engine offload was measured net-negative (shared SBUF ports).
  Measured ~442 us/core steady-state on HW (baseline 580).
"""

import os
import numpy as np
from contextlib import ExitStack

import concourse.mybir as mybir
from concourse.ap import AP

N_CORES = 8

FP16 = mybir.dt.float16
FP32 = mybir.dt.float32
AF = mybir.ActivationFunctionType
ALU = mybir.AluOpType

C = 256
H = W = 128
Q = H * W              # 16384 pixels
G = 2
K = 3
NK = K * K             # 9
NO = G * NK            # 18 conv outputs
PW = W + 2             # padded row pitch (col -1 and 128 reflect)
PR = H + 2             # padded rows (row -1 and 128 reflect)
XPLEN = PR * PW + 2    # 130*130 (+2 tail so kw=2 streams stay in bounds)

SB_ROWS = 16           # super-block rows
N_SB = H // SB_ROWS    # 8
SPAN = SB_ROWS * W     # 2048 compact px per sb (output/DMA)
SP = SB_ROWS * PW      # 2080 pitched px per sb (on-chip sigma/product layout)
CHUNK_ROWS = 2
CHUNK = CHUNK_ROWS * W           # 512 compact px per conv chunk
N_CH = SB_ROWS // CHUNK_ROWS     # 4 conv chunks per sb
CSTREAM = CHUNK_ROWS * PW        # 520 pitched rhs cols per conv chunk
EV = 2 * CSTREAM       # bcast/evac piece: 1040 pitched px


def _mix(cblk):
    """Channel list for mixed tile cblk: 64 group-0 + 64 group-1 channels."""
    lo = [cblk * 64 + i for i in range(64)]
    hi = [128 + cblk * 64 + i for i in range(64)]
    return lo + hi


def host_constants(conv_w, gamma, beta, run_mean, run_var):
    w = np.asarray(conv_w, np.float32)  # (18, 256, 3, 3)
    # narrow conv lhsT: 18 blocks of [128, 18], one per (cblk, kh, kw); the
    # kh/kw shifts ride the rhs stream offset (pitch-130 layout), so all 18
    # matmuls accumulate the combined conv sum z directly in PSUM
    lhsT_conv = np.zeros((128, 18 * NO), np.float16)
    for cb in range(2):
        chans = _mix(cb)
        for kh in range(K):
            for kw in range(K):
                m = (cb * K + kh) * K + kw
                lhsT_conv[:, m * NO:(m + 1) * NO] = w[:, chans, kh, kw].T.astype(np.float16)
    # merged-group selector: per tap k a [18, 128] block;
    # partitions 0:64 take sigma row k (group 0), 64:128 take row 9+k.
    sel = np.zeros((NO, NK * 128), np.float16)
    for k in range(NK):
        sel[k, k * 128:k * 128 + 64] = 1.0
        sel[NK + k, k * 128 + 64:(k + 1) * 128] = 1.0
    ones18 = np.full((NO, NO), 1.0 / 256.0, np.float16)
    scale = np.asarray(gamma, np.float32) / np.sqrt(np.asarray(run_var, np.float32) + 1e-5)
    bias = np.asarray(beta, np.float32) - np.asarray(run_mean, np.float32) * scale
    return {
        "lhsT_conv": lhsT_conv,
        "sel": sel,
        "ones18": ones18,
        "bn_scale": scale.reshape(NO, 1).astype(np.float32),
        "bn_bias": bias.reshape(NO, 1).astype(np.float32),
    }


def declare_io(nc):
    ins = {
        "x": nc.dram_tensor("x", (C, XPLEN), FP16, kind="ExternalInput").ap(),
        "lhsT_conv": nc.dram_tensor("lhsT_conv", (128, 18 * NO), FP16, kind="ExternalInput").ap(),
        "sel": nc.dram_tensor("sel", (NO, NK * 128), FP16, kind="ExternalInput").ap(),
        "ones18": nc.dram_tensor("ones18", (NO, NO), FP16, kind="ExternalInput").ap(),
        "bn_scale": nc.dram_tensor("bn_scale", (NO, 1), FP32, kind="ExternalInput").ap(),
        "bn_bias": nc.dram_tensor("bn_bias", (NO, 1), FP32, kind="ExternalInput").ap(),
    }
    out = nc.dram_tensor("out", (C, N_SB * SP), FP16, kind="ExternalOutput").ap()
    return ins, out


def make_pools(ctx: ExitStack, tc):
    p = {}
    p["const"] = ctx.enter_context(tc.tile_pool(name="const", bufs=1))
    p["xp"] = ctx.enter_context(tc.tile_pool(name="xp", bufs=1))
    p["e"] = ctx.enter_context(tc.tile_pool(name="e", bufs=1))
    p["rchunk"] = ctx.enter_context(tc.tile_pool(name="rchunk", bufs=1))
    p["sgb"] = ctx.enter_context(tc.tile_pool(name="sgb", bufs=2))
    p["pr1"] = ctx.enter_context(tc.tile_pool(name="pr1", bufs=1))
    p["psc"] = ctx.enter_context(tc.tile_pool(name="psc", bufs=2, space="PSUM"))
    p["psb"] = ctx.enter_context(tc.tile_pool(name="psb", bufs=2, space="PSUM"))
    p["psd"] = ctx.enter_context(tc.tile_pool(name="psd", bufs=2, space="PSUM"))
    return p


def load_consts(tc, p, in_aps):
    nc = tc.nc
    const = p["const"]
    c = {}
    for name, shape, dt in (
        ("lhsT_conv", [128, 18 * NO], FP16),
        ("sel", [NO, NK * 128], FP16),
        ("ones18", [NO, NO], FP16),
        ("bn_scale", [NO, 1], FP32),
        ("bn_bias", [NO, 1], FP32),
    ):
        c[name] = const.tile(shape, dt, tag=name, name=name)
        nc.sync.dma_start(c[name][:], in_aps[name][:])
    return c


def emit_body(tc, p, c, out_ap, in_aps):
    nc = tc.nc
    x_d = in_aps["x"]
    lhsT_conv, sel, ones18 = c["lhsT_conv"], c["sel"], c["ones18"]
    bn_scale, bn_bias = c["bn_scale"], c["bn_bias"]

    # ---- x: two channel-mixed pitch-130 padded fp16 tiles ----
    # pos(r, col) = (r+1)*PW + (col+1), r in -1..128, col in -1..128
    xp = []
    for cb in range(2):
        t = p["xp"].tile([128, XPLEN], FP16, tag=f"xp{cb}")
        xp.append(t)
        # host-padded pitch-130 rows: one contiguous DMA per channel block
        nc.sync.dma_start(t[0:64, :], x_d[cb * 64:cb * 64 + 64, :])
        nc.sync.dma_start(t[64:128, :], x_d[128 + cb * 64:128 + cb * 64 + 64, :])

    def emit_conv_chunk(sb, cc, E, den_t):
        """Conv+BN+exp+denominator for 2 rows (520 pitched px).

        The whole sigma pipeline stays in the pitched (row-pitch PW) layout;
        the 2 pad columns per row carry finite garbage that is never DMA'd.
        """
        r0 = sb * SB_ROWS + cc * CHUNK_ROWS
        eq0 = cc * CSTREAM
        cps = p["psc"].tile([NO, CSTREAM], FP32, tag="conv", name="cps")
        for m in range(18):
            cbkh, kw = divmod(m, 3)
            cb, kh = divmod(cbkh, 3)
            base = r0 * PW + kh * PW + kw  # pos(r0+rr-1+kh, col+kw-1) - (rr*PW+col)
            nc.tensor.matmul(
                cps[:],
                lhsT_conv[:, m * NO:(m + 1) * NO],
                xp[cb][:, base:base + CSTREAM],
                start=(m == 0),
                stop=(m == 17),
            )
        # BN + exp -> E chunk (fp16), flat pitched read of conv PSUM
        nc.scalar.activation(E[:, eq0:eq0 + CSTREAM], cps[:], AF.Exp,
                             bias=bn_bias[:], scale=bn_scale[:])
        nc.tensor.matmul(den_t[:], ones18[:],
                         E[:, eq0:eq0 + CSTREAM], start=True, stop=True)
        rch = p["rchunk"].tile([NO, CSTREAM], FP32, tag="r", name="rch")
        with nc.allow_low_precision("softmax recip in fp16"):
            nc.vector.reciprocal_approx_fast(rch[:], den_t[:])
        nc.vector.scalar_tensor_tensor(
            E[:, eq0:eq0 + CSTREAM], E[:, eq0:eq0 + CSTREAM], 1.0 / 256.0,
            rch[:], ALU.mult, ALU.mult,
        )

    def emit_bcast_unit(st, k, piece):
        """Broadcast sigma tap k (both groups) piece -> sgb[k] via PE+ScalarE.

        Each 260-col matmul lands at a bank-aligned 512-fp32 offset in PSUM
        (a matmul output must not cross a 2KB PSUM bank boundary); the evac
        re-packs the two segments contiguously into sgb.
        """
        E, sgb = st["E"], st["sgb"]
        bps = p["psb"].tile([128, 1024], FP32, tag="b", name="bps")
        for j in range(EV // CSTREAM):
            qq = piece * EV + j * CSTREAM
            nc.tensor.matmul(bps[:, j * 512:j * 512 + CSTREAM],
                             sel[:, k * 128:(k + 1) * 128],
                             E[:, qq:qq + CSTREAM], start=True, stop=True)
        b = bps[:]
        src = AP(b.tensor, b.offset, [[b.ap[0][0], 128], [512, 2], [1, CSTREAM]])
        dst = sgb[:, k * SP + piece * EV:k * SP + (piece + 1) * EV]
        nc.scalar.copy(dst.rearrange("p (a b) -> p a b", a=2), src)

    def xvk(cb, sb, k):
        """Flat pitched-sb window of xp[cb] for tap k (contiguous [128, SP])."""
        kh, kw = divmod(k, K)
        base = xp[cb][:]
        pstride = base.ap[0][0]
        return AP(base.tensor, base.offset + (sb * SB_ROWS + kh) * PW + kw,
                  [[pstride, 128], [1, SP]])

    def pool_units(st):
        """Per-tap in-place products (tap k ready as soon as its evacs land),
        then merged 4-op tree adds per tile, then out DMA."""
        sb = st["sb"]
        sgb = st["sgb"]

        def mul(cb, k):
            if cb == 1:
                if "pr1" not in st:
                    st["pr1"] = p["pr1"].tile([128, NK * SP], FP16, tag="pr1",
                                              name="pr1")
                nc.vector.tensor_mul(st["pr1"][:, k * SP:(k + 1) * SP],
                                     sgb[:, k * SP:(k + 1) * SP], xvk(1, sb, k))
            else:
                nc.vector.tensor_mul(sgb[:, k * SP:(k + 1) * SP],
                                     sgb[:, k * SP:(k + 1) * SP], xvk(0, sb, k))

        def tree(which, lvl):
            t = st["pr1"] if which else sgb
            if lvl == 0:
                nc.vector.tensor_add(t[:, 0:4 * SP], t[:, 0:4 * SP],
                                     t[:, 4 * SP:8 * SP])
            elif lvl == 1:
                nc.vector.tensor_add(t[:, 0:2 * SP], t[:, 0:2 * SP],
                                     t[:, 2 * SP:4 * SP])
            elif lvl == 2:
                nc.vector.tensor_add(t[:, 0:SP], t[:, 0:SP], t[:, SP:2 * SP])
            else:
                nc.vector.tensor_add(t[:, 0:SP], t[:, 0:SP], t[:, 8 * SP:9 * SP])

        if os.environ.get("KMOCK_NOPOOL"):  # timing experiment: sigma pipeline only
            yield lambda: emit_out(st, 0)
            yield lambda: emit_out(st, 0)
            return
        # tap 0 last: slot 0 is the tree/out-DMA region of the PREVIOUS sb's
        # pr1 (bufs=1), so touching it first would stall on the out DMA.
        for k in list(range(1, NK)) + [0]:
            yield lambda k=k: mul(1, k)
            yield lambda k=k: mul(0, k)
        for lvl in range(4):
            yield lambda lvl=lvl: tree(1, lvl)
            yield lambda lvl=lvl: tree(0, lvl)
        yield lambda: emit_out(st, 1)
        yield lambda: emit_out(st, 0)

    def emit_out(st, cb):
        """DMA the pooled sb out in the pitched layout (host strips pads)."""
        sb = st["sb"]
        t = st["sgb"] if cb == 0 else st["pr1"]
        q0 = sb * SP
        dst = out_ap.rearrange("(blk grp ch) q -> blk grp ch q", blk=2, grp=2)
        nc.sync.dma_start(dst[0, cb, :, q0:q0 + SP], t[0:64, 0:SP])
        nc.sync.dma_start(dst[1, cb, :, q0:q0 + SP], t[64:128, 0:SP])

    def make_sb_state(sb):
        E = p["e"].tile([NO, SP], FP16, tag="e", name="E")
        sgb = p["sgb"].tile([128, NK * SP], FP16, tag="sgb", name="sgb")
        return {"sb": sb, "E": E, "sgb": sgb}

    # ---- software-pipelined emission over super-blocks ----
    # per sb: Pool chain of prev launches first, then conv chunks + bcast of
    # sb (PE/Act/DVE), then prev's DVE pooling, so Pool and DVE overlap.
    def drain(it, n):
        done = 0
        for fn in it:
            fn()
            done += 1
            if done >= n:
                return
        return

    prev_units = iter(())
    for sb in range(N_SB):
        st = make_sb_state(sb)
        for cc in range(N_CH):
            den_t = p["psd"].tile([NO, CSTREAM], FP32, tag="den", name="den")
            emit_conv_chunk(sb, cc, st["E"], den_t)
            drain(prev_units, 2)
        for k in range(NK):
            for piece in range(SP // EV):
                emit_bcast_unit(st, k, piece)
            drain(prev_units, 1)
        for fn in prev_units:
            fn()
        prev_units = pool_units(st)
    for fn in prev_units:
        fn()


def build(ctx: ExitStack, tc, out_ap, in_aps):
    p = make_pools(ctx, tc)
    c = load_consts(tc, p, in_aps)
    emit_body(tc, p, c, out_ap, in_aps)


_COMPILED = {}


def _get_compiled():
    if "nc" not in _COMPILED:
        import concourse.bacc as bacc
        import concourse.tile as tile

        nc = bacc.Bacc("TRN2", target_bir_lowering=False, debug=False,
                       num_devices=N_CORES)
        ins, out_ap = declare_io(nc)
        with tile.TileContext(nc) as tc:
            with ExitStack() as ctx:
                build(ctx, tc, out_ap, ins)
        nc.compile()
        _COMPILED["nc"] = nc
    return _COMPILED["nc"]


def host_x(x_sample):
    """Reflect-pad one sample to the pitch-130 on-chip layout (fp16)."""
    xs = np.asarray(x_sample, np.float32).reshape(C, H, W)
    xpad = np.pad(xs, ((0, 0), (1, 1), (1, 1)), mode="reflect")
    flat = np.zeros((C, XPLEN), np.float16)
    flat[:, :PR * PW] = xpad.astype(np.float16).reshape(C, PR * PW)
    return flat


def kernel(x, conv_w, gamma, beta, run_mean, run_var):
    from concourse import bass_utils

    x = np.asarray(x, np.float32)
    n = x.shape[0]
    assert n == N_CORES, f"expected batch {N_CORES}, got {n}"
    consts = host_constants(np.asarray(conv_w, np.float32), np.asarray(gamma, np.float32),
                            np.asarray(beta, np.float32), np.asarray(run_mean, np.float32),
                            np.asarray(run_var, np.float32))
    nc = _get_compiled()
    in_maps = [{"x": host_x(x[i]), **consts} for i in range(N_CORES)]
    res = bass_utils.run_bass_kernel_spmd(nc, in_maps, core_ids=list(range(N_CORES)))
    out = np.stack([
        res.results[i]["out"].reshape(C, N_SB, SB_ROWS, PW)[:, :, :, 0:W]
        .reshape(C, H, W)
        for i in range(N_CORES)
    ])
    return out.astype(np.float32)

